# revision 13
# baseline (speedup 1.0000x reference)
"""Trainium2 Bass kernel for nn_AttentionLayer (dense_mlp, 8-core data parallel).

Reference computation (per batch b of 2048, S=200 steps, E=128):
    feat[b,s] = concat(x, t, x*t, x-t)            # [4E] with x=behaviors[b,s], t=target[b]
    h = relu(feat @ W1 + b1)                      # [64]
    w = sigmoid(h @ W2 + b2)                      # scalar
    out[b]   = sum_s w[b,s] * x[b,s]              # [128]

Host-side algebra (weights + per-batch folds, all tiny):
    feat @ W1 = x @ (W1a + W1d + t_col*W1c) + t @ (W1b - W1d)
      Wb_b  = W1a + W1d + t_b[:,None]*W1c        # per-batch [E,A] weight
      csb_b = t_b @ (W1b - W1d) + b1             # per-batch [A] bias

Device dataflow (per core, 256 batches, all matmul operands bf16).
Host uploads x in BOTH layouts so the device never transposes:
    xt    [E, b*S]   : mm1 moving operand
    nat_a [s0:128, b*E], nat_b [s128:200, b*E] : po moving operand
    wb    [E, b*A]   : per-batch folded mm1 weights (stationary)
Per 2-batch group g (batches 2g, 2g+1), software-pipelined with lags so
no in-order queue ever waits on a same-iteration producer:
  i+0  PE : ph[64j:+64, 0:200] = wb_bj.T @ xt_bj      (col tiles 0/64)
  i+1  DVE: hs [128,200] bf16 = max(ph + csb2[:,g], 0)
  i+2  PE : pw[0:128, 4q:+2] = hs[:,0:128].T @ w2s    (w2s=[[W2,0],[0,W2]],
            pw[0:72, 4q+2:+2] = hs[:,128:200].T @ w2s  both batches at once;
            pw tile spans a PAIR of groups, q = g%2)
  i+3  ACT: ws [128,8] bf16 = sigmoid(pw + b2) once per pair
  i+4  PE : po[32*slot:+1, 0:128] = ws_cA.T @ nat_a + ws_cB.T @ nat_b
            (psum row per batch at partitions {0,32,64,96}; [b,e] layout)
  i+6  DVE: osb half = copy(po pair tile [128,128])
  i+8  DMA: 8 output rows (2 pairs) sbuf -> DRAM, partition-strided AP
"""

import sys

sys.path.insert(0, "/opt/trn_rl_repo")

import numpy as np
import ml_dtypes

import concourse.bass as bass
import concourse.mybir as mybir
from concourse.tile import TileContext
from concourse.bass_utils import run_bass_kernel_spmd

F32 = mybir.dt.float32
BF16 = mybir.dt.bfloat16
AF = mybir.ActivationFunctionType
ALU = mybir.AluOpType

B, S, E, A = 2048, 200, 128, 64
NCORES = 8
BL = B // NCORES  # 256 batches per core
G = 2  # batches per group (stacked in partition halves)
NG = BL // G  # 128 groups
DG = 16  # batches per DMA granule
GPG = DG // G  # groups per granule (8)
NDG = BL // DG  # 16 granules
SA, SB = 128, S - 128  # s-chunk sizes (128 + 72)


def build_graph() -> bass.Bass:
    nc = bass.Bass()

    xt_d = nc.declare_dram_parameter("xt", [E, BL * S], BF16, isOutput=False)
    na_d = nc.declare_dram_parameter("nat_a", [SA, BL * E], BF16, isOutput=False)
    nb_d = nc.declare_dram_parameter("nat_b", [SB, BL * E], BF16, isOutput=False)
    wb_d = nc.declare_dram_parameter("wb", [E, BL * A], BF16, isOutput=False)
    w2s_d = nc.declare_dram_parameter("w2s", [128, 2], BF16, isOutput=False)
    b2c_d = nc.declare_dram_parameter("b2c", [128, 1], F32, isOutput=False)
    csb2_d = nc.declare_dram_parameter("csb2", [128, NG], F32, isOutput=False)
    out_d = nc.declare_dram_parameter("out", [BL, E], F32, isOutput=True)

    with TileContext(nc) as tc:
        with (
            tc.tile_pool(name="consts", bufs=1) as cpool,
            tc.tile_pool(name="xtp", bufs=3) as xtpool,
            tc.tile_pool(name="nap", bufs=3) as napool,
            tc.tile_pool(name="nbp", bufs=3) as nbpool,
            tc.tile_pool(name="wbp", bufs=3) as wbpool,
            tc.tile_pool(name="hs", bufs=3) as hspool,
            tc.tile_pool(name="ws", bufs=3) as wspool,
            tc.tile_pool(name="osb", bufs=3) as osbpool,
            tc.tile_pool(name="ph", bufs=3, space="PSUM") as php,
            tc.tile_pool(name="pw", bufs=2, space="PSUM") as pwp,
            tc.tile_pool(name="po", bufs=3, space="PSUM") as pop,
        ):
            w2s = cpool.tile([128, 2], BF16)
            b2c = cpool.tile([128, 1], F32)
            csb2 = cpool.tile([128, NG], F32)
            nc.sync.dma_start(out=w2s[:], in_=w2s_d[:])
            nc.sync.dma_start(out=b2c[:], in_=b2c_d[:])
            nc.sync.dma_start(out=csb2[:], in_=csb2_d[:])

            gran = {}  # dg -> (xtt, nat, nbt, wbt)
            phs = {}  # g -> ph tile
            hss = {}  # g -> hs tile
            pws = {}  # pair -> pw tile
            wss = {}  # pair -> ws tile
            pos = {}  # pair -> po tile
            osbs = {}  # k -> osb tile (8 batches)

            for i in range(NG + 8):
                # -- granule DMA loads (16 batches ahead of use) --
                if i % GPG == 0 and i < NG:
                    dg = i // GPG
                    xtt = xtpool.tile([E, DG * S], BF16, tag="xt")
                    nat = napool.tile([SA, DG * E], BF16, tag="na")
                    nbt = nbpool.tile([SB, DG * E], BF16, tag="nb")
                    wbt = wbpool.tile([E, DG * A], BF16, tag="wb")
                    sl = slice(dg * DG * S, (dg + 1) * DG * S)
                    se = slice(dg * DG * E, (dg + 1) * DG * E)
                    sa = slice(dg * DG * A, (dg + 1) * DG * A)
                    # round-robin the 4 streams over the 3 DMA-capable queues
                    qs = [nc.gpsimd, nc.sync, nc.scalar]
                    r = dg % 3
                    qs[r].dma_start(out=xtt[:], in_=xt_d[:, sl])
                    qs[(r + 1) % 3].dma_start(out=nat[:], in_=na_d[:, se])
                    qs[(r + 2) % 3].dma_start(out=nbt[:], in_=nb_d[:, se])
                    qs[r].dma_start(out=wbt[:], in_=wb_d[:, sa])
                    gran[dg] = (xtt, nat, nbt, wbt)

                # -- PE: mm1(i) --
                if i < NG:
                    g = i
                    xtt, _, _, wbt = gran[g // GPG]
                    ph = php.tile([128, S], mybir.dt.float32, tag="ph")
                    for j in range(G):
                        b = (g % GPG) * G + j  # batch idx within granule
                        nc.tensor.matmul(
                            ph[64 * j : 64 * j + 64, :],
                            wbt[:, b * A : (b + 1) * A],
                            xtt[:, b * S : (b + 1) * S],
                            start=True,
                            stop=True,
                        )
                    phs[g] = ph

                # -- relu(i-1): alternate DVE / ACT to balance the engines --
                if 0 <= i - 1 < NG:
                    g = i - 1
                    hs = hspool.tile([128, S], BF16, tag="hs")
                    if g % 2 == 0:
                        nc.vector.tensor_scalar(
                            hs[:], phs.pop(g)[:], csb2[:, g : g + 1], 0.0,
                            op0=ALU.add, op1=ALU.max,
                        )
                    else:
                        nc.scalar.activation(
                            hs[:], phs.pop(g)[:], AF.Relu,
                            bias=csb2[:, g : g + 1], scale=1.0,
                        )
                    hss[g] = hs

                # -- PE: pw(i-2), pair-merged psum tile --
                if 0 <= i - 2 < NG:
                    g = i - 2
                    p, q = g // 2, g % 2
                    if q == 0:
                        pws[p] = pwp.tile([128, 8], mybir.dt.float32, tag="pw", name=f"pw{p}")
                    pw = pws[p]
                    hs = hss.pop(g)
                    nc.tensor.matmul(
                        pw[0:128, 4 * q : 4 * q + 2], hs[:, 0:SA], w2s[:],
                        start=True, stop=True,
                    )
                    nc.tensor.matmul(
                        pw[0:SB, 4 * q + 2 : 4 * q + 4], hs[:, SA:S], w2s[:],
                        start=True, stop=True,
                    )

                # -- ACT: sigmoid per pair, after pw of odd group --
                if 0 <= i - 3 < NG and (i - 3) % 2 == 1:
                    p = (i - 3) // 2
                    ws = wspool.tile([128, 8], BF16, tag="ws")
                    nc.scalar.activation(
                        ws[:], pws.pop(p)[:], AF.Sigmoid, bias=b2c[:, 0:1], scale=1.0
                    )
                    wss[p] = ws

                # -- PE: po(i-4), 4 batches per psum tile: partition {0,64} x col half --
                if 0 <= i - 4 < NG:
                    g = i - 4
                    p, q = g // 2, g % 2
                    m, c = g // 2, g % 2  # po-tile index, column half
                    if c == 0:
                        pos[m] = pop.tile([128, 256], mybir.dt.float32, tag="po", name=f"po{m}")
                    po = pos[m]
                    ws = wss[p]
                    _, nat, nbt, _ = gran[g // GPG]
                    for j in range(G):
                        b = (g % GPG) * G + j
                        nc.tensor.matmul(
                            po[64 * j : 64 * j + 1, c * E : c * E + E],
                            ws[0:SA, 4 * q + j : 4 * q + j + 1],
                            nat[:, b * E : (b + 1) * E],
                            start=True,
                            stop=False,
                        )
                        nc.tensor.matmul(
                            po[64 * j : 64 * j + 1, c * E : c * E + E],
                            ws[0:SB, 4 * q + 2 + j : 4 * q + 3 + j],
                            nbt[:, b * E : (b + 1) * E],
                            start=False,
                            stop=True,
                        )
                    if q == 1:
                        wss.pop(p)

                # -- DVE: drain po tile (4 batches) into osb half --
                if 0 <= i - 6 < NG and (i - 6) % 2 == 1:
                    m = (i - 6) // 2
                    k, half = m // 2, m % 2
                    if half == 0:
                        osbs[k] = osbpool.tile([128, 512], mybir.dt.float32, tag="osb", name=f"osb{k}")
                    nc.vector.tensor_copy(
                        osbs[k][:, half * 256 : half * 256 + 256], pos.pop(m)[:]
                    )

                # -- out DMA: 8 rows per osb tile (2 po tiles) --
                if i >= 10 and (i - 10) % 4 == 0 and (i - 10) // 4 * 8 < BL:
                    k = (i - 10) // 4
                    r0 = k * 8
                    # sbuf [2 part (stride 64), mm, c, e] <-> dram row r=4mm+2c+j
                    dst = out_d[r0 : r0 + 8, :].rearrange(
                        "(mm c j) e -> j mm c e", mm=2, c=2, j=2
                    )
                    nc.scalar.dma_start(out=dst, in_=osbs.pop(k)[0:128:64, :])
    _hoist_excess_waits(nc)
    return nc


# Instructions on engine queues accept only ONE sync-wait command in this
# toolchain (walrus setupSyncWait). Tile's sem assigner sometimes attaches
# more. Hoist the excess onto same-engine NoOps inserted immediately before
# the instruction — identical semantics, the wait just moves one queue slot
# earlier. DMA/Drain/branch instructions are exempt (different lowering).
_WAIT_CAP_EXEMPT = {"InstNoOp"}


def _hoist_excess_waits(nc) -> int:
    k = 0
    for fn in nc.m.functions:
        for bb in fn.blocks:
            il = bb.instructions
            out = []
            changed = False
            for inst in il:
                si = inst.sync_info
                tn = type(inst).__name__
                if si is not None and len(si.on_wait) > 1 and tn not in _WAIT_CAP_EXEMPT:
                    waits = list(si.on_wait)
                    for w in waits[:-1]:
                        nop = mybir.InstNoOp(name=f"W-hoist-{k}")
                        k += 1
                        nop.engine = inst.engine
                        nop.sync_info = mybir.SyncInfo(on_wait=[w], on_update=[])
                        out.append(nop)
                    inst.sync_info = mybir.SyncInfo(
                        on_wait=[waits[-1]], on_update=list(si.on_update)
                    )
                    changed = True
                out.append(inst)
            if changed:
                bb.instructions = out
    return k


_GRAPH_CACHE: dict = {}

# test-harness hooks (harness calls kernel() with defaults; test.py flips TRACE)
TRACE = False
TRACE_TMPDIR = None
LAST_RESULT = None


def kernel(**inputs) -> np.ndarray:
    BF = ml_dtypes.bfloat16
    behaviors = np.asarray(inputs["behaviors"], dtype=np.float32)
    target = np.asarray(inputs["target"], dtype=np.float32)
    W1 = np.asarray(inputs["W1"], dtype=np.float32)
    b1 = np.asarray(inputs["b1"], dtype=np.float32)
    W2 = np.asarray(inputs["W2"], dtype=np.float32)
    b2 = np.asarray(inputs["b2"], dtype=np.float32)

    W1a, W1b, W1c, W1d = W1[0:E], W1[E : 2 * E], W1[2 * E : 3 * E], W1[3 * E :]
    W1ad = W1a + W1d  # [E, A]
    W1bd = W1b - W1d  # [E, A]
    b2f = float(np.asarray(b2).reshape(-1)[0])

    if "nc" not in _GRAPH_CACHE:
        _GRAPH_CACHE["nc"] = build_graph()
    nc = _GRAPH_CACHE["nc"]

    x = behaviors.reshape(NCORES, BL, S, E)
    t = target.reshape(NCORES, BL, E)

    # w2s: [[W2, 0], [0, W2]] so one matmul computes both stacked batches
    w2s = np.zeros((128, 2), dtype=np.float32)
    w2s[0:A, 0] = W2[:, 0]
    w2s[A:128, 1] = W2[:, 0]
    w2s = w2s.astype(BF)
    b2c = np.full((128, 1), b2f, dtype=np.float32)

    in_maps = []
    for i in range(NCORES):
        xi = x[i]  # [BL, S, E] f32
        ti = t[i]  # [BL, E]
        xt = np.ascontiguousarray(xi.transpose(2, 0, 1)).astype(BF).reshape(E, BL * S)
        nat = np.ascontiguousarray(xi.transpose(1, 0, 2)).astype(BF)  # [S, BL, E]
        na = nat[0:SA].reshape(SA, BL * E)
        nb = nat[SA:S].reshape(SB, BL * E)
        # per-batch folded weight: Wb = W1ad + t_col * W1c  -> [E, BL*A]
        wb = W1ad[None, :, :] + ti[:, :, None] * W1c[None, :, :]  # [BL, E, A]
        wb = np.ascontiguousarray(wb.transpose(1, 0, 2)).astype(BF).reshape(E, BL * A)
        # per-batch bias, stacked per group: csb2[j*64+a, g] = csb[2g+j, a]
        csb = ti @ W1bd + b1[None, :]  # [BL, A] f32
        csb2 = np.ascontiguousarray(
            csb.reshape(NG, G, A).transpose(1, 2, 0).reshape(128, NG)
        )
        in_maps.append(
            dict(xt=xt, nat_a=na, nat_b=nb, wb=wb, w2s=w2s, b2c=b2c, csb2=csb2)
        )

    global LAST_RESULT
    kw = {}
    if TRACE:
        kw = dict(trace=True, tmpdir=TRACE_TMPDIR)
    res = run_bass_kernel_spmd(nc, in_maps, core_ids=list(range(NCORES)), **kw)
    LAST_RESULT = res
    out = np.stack([res.results[i]["out"] for i in range(NCORES)], axis=0)
    return out.reshape(B, E).astype(np.float32)


if __name__ == "__main__":
    rng = np.random.default_rng(0)
    ins = dict(
        behaviors=rng.standard_normal((B, S, E), dtype=np.float32),
        target=rng.standard_normal((B, E), dtype=np.float32),
        W1=rng.standard_normal((4 * E, A), dtype=np.float32) * 0.04,
        b1=rng.standard_normal((A,), dtype=np.float32) * 0.04,
        W2=rng.standard_normal((A, 1), dtype=np.float32) * 0.1,
        b2=rng.standard_normal((1,), dtype=np.float32) * 0.1,
    )
    o = kernel(**ins)
    print("kernel out", o.shape, o.dtype, np.abs(o).mean())


# revision 15
# speedup vs baseline: 1.1033x; 1.1033x over previous
"""Trainium2 Bass kernel for nn_AttentionLayer (dense_mlp, 8-core data parallel).

Reference computation (per batch b of 2048, S=200 steps, E=128):
    feat[b,s] = concat(x, t, x*t, x-t)            # [4E] with x=behaviors[b,s], t=target[b]
    h = relu(feat @ W1 + b1)                      # [64]
    w = sigmoid(h @ W2 + b2)                      # scalar
    out[b]   = sum_s w[b,s] * x[b,s]              # [128]

Host-side algebra (weights + per-batch folds, all tiny):
    feat @ W1 = x @ (W1a + W1d + t_col*W1c) + t @ (W1b - W1d)
      Wb_b  = W1a + W1d + t_b[:,None]*W1c        # per-batch [E,A] weight
      csb_b = t_b @ (W1b - W1d) + b1             # per-batch [A] bias

Device dataflow (per core, 256 batches, all matmul operands bf16).
Host uploads x in BOTH layouts so the device never transposes:
    xt    [E, b*S]   : mm1 moving operand
    nat_a [s0:128, b*E], nat_b [s128:200, b*E] : po moving operand
    wb    [E, b*A]   : per-batch folded mm1 weights (stationary)
Per 2-batch group g (batches 2g, 2g+1), software-pipelined with lags so
no in-order queue ever waits on a same-iteration producer:
  i+0  PE : ph[64j:+64, 0:200] = wb_bj.T @ xt_bj      (col tiles 0/64)
  i+1  DVE: hs [128,200] bf16 = max(ph + csb2[:,g], 0)
  i+2  PE : pw[0:128, 4q:+2] = hs[:,0:128].T @ w2s    (w2s=[[W2,0],[0,W2]],
            pw[0:72, 4q+2:+2] = hs[:,128:200].T @ w2s  both batches at once;
            pw tile spans a PAIR of groups, q = g%2)
  i+3  ACT: ws [128,8] bf16 = sigmoid(pw + b2) once per pair
  i+4  PE : po[32*slot:+1, 0:128] = ws_cA.T @ nat_a + ws_cB.T @ nat_b
            (psum row per batch at partitions {0,32,64,96}; [b,e] layout)
  i+6  DVE: osb half = copy(po pair tile [128,128])
  i+8  DMA: 8 output rows (2 pairs) sbuf -> DRAM, partition-strided AP
"""

import sys

sys.path.insert(0, "/opt/trn_rl_repo")

import numpy as np
import ml_dtypes

import concourse.bass as bass
import concourse.mybir as mybir
from concourse.tile import TileContext
from concourse.bass_utils import run_bass_kernel_spmd

F32 = mybir.dt.float32
BF16 = mybir.dt.bfloat16
AF = mybir.ActivationFunctionType
ALU = mybir.AluOpType

B, S, E, A = 2048, 200, 128, 64
NCORES = 8
BL = B // NCORES  # 256 batches per core
G = 2  # batches per group (stacked in partition halves)
NG = BL // G  # 128 groups
DG = 16  # batches per DMA granule
GPG = DG // G  # groups per granule (8)
NDG = BL // DG  # 16 granules
SA, SB = 128, S - 128  # s-chunk sizes (128 + 72)


def build_graph() -> bass.Bass:
    nc = bass.Bass()

    xt_d = nc.declare_dram_parameter("xt", [E, BL * S], BF16, isOutput=False)
    na_d = nc.declare_dram_parameter("nat_a", [SA, BL * E], BF16, isOutput=False)
    nb_d = nc.declare_dram_parameter("nat_b", [SB, BL * E], BF16, isOutput=False)
    wb_d = nc.declare_dram_parameter("wb", [E, BL * A], BF16, isOutput=False)
    w2s_d = nc.declare_dram_parameter("w2s", [128, 2], BF16, isOutput=False)
    b2c_d = nc.declare_dram_parameter("b2c", [128, 1], F32, isOutput=False)
    csb2_d = nc.declare_dram_parameter("csb2", [128, NG], F32, isOutput=False)
    out_d = nc.declare_dram_parameter("out", [BL, E], F32, isOutput=True)

    with TileContext(nc) as tc:
        with (
            tc.tile_pool(name="consts", bufs=1) as cpool,
            tc.tile_pool(name="xtp", bufs=3) as xtpool,
            tc.tile_pool(name="nap", bufs=3) as napool,
            tc.tile_pool(name="nbp", bufs=3) as nbpool,
            tc.tile_pool(name="wbp", bufs=3) as wbpool,
            tc.tile_pool(name="hs", bufs=3) as hspool,
            tc.tile_pool(name="ws", bufs=3) as wspool,
            tc.tile_pool(name="osb", bufs=3) as osbpool,
            tc.tile_pool(name="ph", bufs=3, space="PSUM") as php,
            tc.tile_pool(name="pw", bufs=2, space="PSUM") as pwp,
            tc.tile_pool(name="po", bufs=3, space="PSUM") as pop,
        ):
            w2s = cpool.tile([128, 2], BF16)
            b2c = cpool.tile([128, 1], F32)
            csb2 = cpool.tile([128, NG], F32)
            nc.sync.dma_start(out=w2s[:], in_=w2s_d[:])
            nc.sync.dma_start(out=b2c[:], in_=b2c_d[:])
            nc.sync.dma_start(out=csb2[:], in_=csb2_d[:])

            gran = {}  # dg -> (xtt, nat, nbt, wbt)
            phs = {}  # g -> ph tile
            hss = {}  # g -> hs tile
            pws = {}  # pair -> pw tile
            wss = {}  # pair -> ws tile
            pos = {}  # pair -> po tile
            osbs = {}  # k -> osb tile (8 batches)

            for i in range(NG + 8):
                # -- granule DMA loads (16 batches ahead of use) --
                if i % GPG == 0 and i < NG:
                    dg = i // GPG
                    xtt = xtpool.tile([E, DG * S], BF16, tag="xt")
                    nat = napool.tile([SA, DG * E], BF16, tag="na")
                    nbt = nbpool.tile([SB, DG * E], BF16, tag="nb")
                    wbt = wbpool.tile([E, DG * A], BF16, tag="wb")
                    sl = slice(dg * DG * S, (dg + 1) * DG * S)
                    se = slice(dg * DG * E, (dg + 1) * DG * E)
                    sa = slice(dg * DG * A, (dg + 1) * DG * A)
                    # DMA only on the compute-free queues (gpsimd + sync):
                    # a DMA stalled on pool-WAR must never block relu/sigmoid
                    nc.gpsimd.dma_start(out=xtt[:], in_=xt_d[:, sl])
                    nc.sync.dma_start(out=nat[:], in_=na_d[:, se])
                    nc.gpsimd.dma_start(out=nbt[:], in_=nb_d[:, se])
                    nc.sync.dma_start(out=wbt[:], in_=wb_d[:, sa])
                    gran[dg] = (xtt, nat, nbt, wbt)

                # -- PE: mm1(i) --
                if i < NG:
                    g = i
                    xtt, _, _, wbt = gran[g // GPG]
                    ph = php.tile([128, S], mybir.dt.float32, tag="ph")
                    for j in range(G):
                        b = (g % GPG) * G + j  # batch idx within granule
                        nc.tensor.matmul(
                            ph[64 * j : 64 * j + 64, :],
                            wbt[:, b * A : (b + 1) * A],
                            xtt[:, b * S : (b + 1) * S],
                            start=True,
                            stop=True,
                        )
                    phs[g] = ph

                # -- relu(i-1): alternate DVE / ACT to balance the engines --
                if 0 <= i - 1 < NG:
                    g = i - 1
                    hs = hspool.tile([128, S], BF16, tag="hs")
                    if g % 2 == 0:
                        nc.vector.tensor_scalar(
                            hs[:], phs.pop(g)[:], csb2[:, g : g + 1], 0.0,
                            op0=ALU.add, op1=ALU.max,
                        )
                    else:
                        nc.scalar.activation(
                            hs[:], phs.pop(g)[:], AF.Relu,
                            bias=csb2[:, g : g + 1], scale=1.0,
                        )
                    hss[g] = hs

                # -- PE: pw(i-2), pair-merged psum tile --
                if 0 <= i - 2 < NG:
                    g = i - 2
                    p, q = g // 2, g % 2
                    if q == 0:
                        pws[p] = pwp.tile([128, 8], mybir.dt.float32, tag="pw", name=f"pw{p}")
                    pw = pws[p]
                    hs = hss.pop(g)
                    nc.tensor.matmul(
                        pw[0:128, 4 * q : 4 * q + 2], hs[:, 0:SA], w2s[:],
                        start=True, stop=True,
                    )
                    nc.tensor.matmul(
                        pw[0:SB, 4 * q + 2 : 4 * q + 4], hs[:, SA:S], w2s[:],
                        start=True, stop=True,
                    )

                # -- ACT: sigmoid per pair, after pw of odd group --
                if 0 <= i - 3 < NG and (i - 3) % 2 == 1:
                    p = (i - 3) // 2
                    ws = wspool.tile([128, 8], BF16, tag="ws")
                    nc.scalar.activation(
                        ws[:], pws.pop(p)[:], AF.Sigmoid, bias=b2c[:, 0:1], scale=1.0
                    )
                    wss[p] = ws

                # -- PE: po(i-4), 4 batches per psum tile: partition {0,64} x col half --
                if 0 <= i - 4 < NG:
                    g = i - 4
                    p, q = g // 2, g % 2
                    m, c = g // 2, g % 2  # po-tile index, column half
                    if c == 0:
                        pos[m] = pop.tile([128, 256], mybir.dt.float32, tag="po", name=f"po{m}")
                    po = pos[m]
                    ws = wss[p]
                    _, nat, nbt, _ = gran[g // GPG]
                    for j in range(G):
                        b = (g % GPG) * G + j
                        nc.tensor.matmul(
                            po[64 * j : 64 * j + 1, c * E : c * E + E],
                            ws[0:SA, 4 * q + j : 4 * q + j + 1],
                            nat[:, b * E : (b + 1) * E],
                            start=True,
                            stop=False,
                        )
                        nc.tensor.matmul(
                            po[64 * j : 64 * j + 1, c * E : c * E + E],
                            ws[0:SB, 4 * q + 2 + j : 4 * q + 3 + j],
                            nbt[:, b * E : (b + 1) * E],
                            start=False,
                            stop=True,
                        )
                    if q == 1:
                        wss.pop(p)

                # -- DVE: drain po tile (4 batches) into osb half --
                if 0 <= i - 6 < NG and (i - 6) % 2 == 1:
                    m = (i - 6) // 2
                    k, half = m // 2, m % 2
                    if half == 0:
                        osbs[k] = osbpool.tile([128, 512], mybir.dt.float32, tag="osb", name=f"osb{k}")
                    nc.vector.tensor_copy(
                        osbs[k][:, half * 256 : half * 256 + 256], pos.pop(m)[:]
                    )

                # -- out DMA: 8 rows per osb tile (2 po tiles) --
                if i >= 10 and (i - 10) % 4 == 0 and (i - 10) // 4 * 8 < BL:
                    k = (i - 10) // 4
                    r0 = k * 8
                    # sbuf [2 part (stride 64), mm, c, e] <-> dram row r=4mm+2c+j
                    dst = out_d[r0 : r0 + 8, :].rearrange(
                        "(mm c j) e -> j mm c e", mm=2, c=2, j=2
                    )
                    nc.gpsimd.dma_start(out=dst, in_=osbs.pop(k)[0:128:64, :])
    _hoist_excess_waits(nc)
    return nc


# Instructions on engine queues accept only ONE sync-wait command in this
# toolchain (walrus setupSyncWait). Tile's sem assigner sometimes attaches
# more. Hoist the excess onto same-engine NoOps inserted immediately before
# the instruction — identical semantics, the wait just moves one queue slot
# earlier. DMA/Drain/branch instructions are exempt (different lowering).
_WAIT_CAP_EXEMPT = {"InstNoOp"}


def _hoist_excess_waits(nc) -> int:
    k = 0
    for fn in nc.m.functions:
        for bb in fn.blocks:
            il = bb.instructions
            out = []
            changed = False
            for inst in il:
                si = inst.sync_info
                tn = type(inst).__name__
                if si is not None and len(si.on_wait) > 1 and tn not in _WAIT_CAP_EXEMPT:
                    waits = list(si.on_wait)
                    for w in waits[:-1]:
                        nop = mybir.InstNoOp(name=f"W-hoist-{k}")
                        k += 1
                        nop.engine = inst.engine
                        nop.sync_info = mybir.SyncInfo(on_wait=[w], on_update=[])
                        out.append(nop)
                    inst.sync_info = mybir.SyncInfo(
                        on_wait=[waits[-1]], on_update=list(si.on_update)
                    )
                    changed = True
                out.append(inst)
            if changed:
                bb.instructions = out
    return k


_GRAPH_CACHE: dict = {}

# test-harness hooks (harness calls kernel() with defaults; test.py flips TRACE)
TRACE = False
TRACE_TMPDIR = None
LAST_RESULT = None


def kernel(**inputs) -> np.ndarray:
    BF = ml_dtypes.bfloat16
    behaviors = np.asarray(inputs["behaviors"], dtype=np.float32)
    target = np.asarray(inputs["target"], dtype=np.float32)
    W1 = np.asarray(inputs["W1"], dtype=np.float32)
    b1 = np.asarray(inputs["b1"], dtype=np.float32)
    W2 = np.asarray(inputs["W2"], dtype=np.float32)
    b2 = np.asarray(inputs["b2"], dtype=np.float32)

    W1a, W1b, W1c, W1d = W1[0:E], W1[E : 2 * E], W1[2 * E : 3 * E], W1[3 * E :]
    W1ad = W1a + W1d  # [E, A]
    W1bd = W1b - W1d  # [E, A]
    b2f = float(np.asarray(b2).reshape(-1)[0])

    if "nc" not in _GRAPH_CACHE:
        _GRAPH_CACHE["nc"] = build_graph()
    nc = _GRAPH_CACHE["nc"]

    x = behaviors.reshape(NCORES, BL, S, E)
    t = target.reshape(NCORES, BL, E)

    # w2s: [[W2, 0], [0, W2]] so one matmul computes both stacked batches
    w2s = np.zeros((128, 2), dtype=np.float32)
    w2s[0:A, 0] = W2[:, 0]
    w2s[A:128, 1] = W2[:, 0]
    w2s = w2s.astype(BF)
    b2c = np.full((128, 1), b2f, dtype=np.float32)

    in_maps = []
    for i in range(NCORES):
        xi = x[i]  # [BL, S, E] f32
        ti = t[i]  # [BL, E]
        xt = np.ascontiguousarray(xi.transpose(2, 0, 1)).astype(BF).reshape(E, BL * S)
        nat = np.ascontiguousarray(xi.transpose(1, 0, 2)).astype(BF)  # [S, BL, E]
        na = nat[0:SA].reshape(SA, BL * E)
        nb = nat[SA:S].reshape(SB, BL * E)
        # per-batch folded weight: Wb = W1ad + t_col * W1c  -> [E, BL*A]
        wb = W1ad[None, :, :] + ti[:, :, None] * W1c[None, :, :]  # [BL, E, A]
        wb = np.ascontiguousarray(wb.transpose(1, 0, 2)).astype(BF).reshape(E, BL * A)
        # per-batch bias, stacked per group: csb2[j*64+a, g] = csb[2g+j, a]
        csb = ti @ W1bd + b1[None, :]  # [BL, A] f32
        csb2 = np.ascontiguousarray(
            csb.reshape(NG, G, A).transpose(1, 2, 0).reshape(128, NG)
        )
        in_maps.append(
            dict(xt=xt, nat_a=na, nat_b=nb, wb=wb, w2s=w2s, b2c=b2c, csb2=csb2)
        )

    global LAST_RESULT
    kw = {}
    if TRACE:
        kw = dict(trace=True, tmpdir=TRACE_TMPDIR)
    res = run_bass_kernel_spmd(nc, in_maps, core_ids=list(range(NCORES)), **kw)
    LAST_RESULT = res
    out = np.stack([res.results[i]["out"] for i in range(NCORES)], axis=0)
    return out.reshape(B, E).astype(np.float32)


if __name__ == "__main__":
    rng = np.random.default_rng(0)
    ins = dict(
        behaviors=rng.standard_normal((B, S, E), dtype=np.float32),
        target=rng.standard_normal((B, E), dtype=np.float32),
        W1=rng.standard_normal((4 * E, A), dtype=np.float32) * 0.04,
        b1=rng.standard_normal((A,), dtype=np.float32) * 0.04,
        W2=rng.standard_normal((A, 1), dtype=np.float32) * 0.1,
        b2=rng.standard_normal((1,), dtype=np.float32) * 0.1,
    )
    o = kernel(**ins)
    print("kernel out", o.shape, o.dtype, np.abs(o).mean())


# revision 18
# speedup vs baseline: 1.1841x; 1.0732x over previous
"""Trainium2 Bass kernel for nn_AttentionLayer (dense_mlp, 8-core data parallel).

Reference computation (per batch b of 2048, S=200 steps, E=128):
    feat[b,s] = concat(x, t, x*t, x-t)            # [4E] with x=behaviors[b,s], t=target[b]
    h = relu(feat @ W1 + b1)                      # [64]
    w = sigmoid(h @ W2 + b2)                      # scalar
    out[b]   = sum_s w[b,s] * x[b,s]              # [128]

Host-side algebra (weights + per-batch folds, all tiny):
    feat @ W1 = x @ (W1a + W1d + t_col*W1c) + t @ (W1b - W1d)
      Wb_b  = W1a + W1d + t_b[:,None]*W1c        # per-batch [E,A] weight
      csb_b = t_b @ (W1b - W1d) + b1             # per-batch [A] bias

Device dataflow (per core, 256 batches, all matmul operands bf16).
Host uploads x in BOTH layouts so the device never transposes:
    xt    [E, b*S]   : mm1 moving operand
    nat_a [s0:128, b*E], nat_b [s128:200, b*E] : po moving operand
    wb    [E, b*A]   : per-batch folded mm1 weights (stationary)
Per 2-batch group g (batches 2g, 2g+1), software-pipelined with lags so
no in-order queue ever waits on a same-iteration producer:
  i+0  PE : ph[64j:+64, 0:200] = wb_bj.T @ xt_bj      (col tiles 0/64)
  i+1  DVE: hs [128,200] bf16 = max(ph + csb2[:,g], 0)
  i+2  PE : pw[0:128, 4q:+2] = hs[:,0:128].T @ w2s    (w2s=[[W2,0],[0,W2]],
            pw[0:72, 4q+2:+2] = hs[:,128:200].T @ w2s  both batches at once;
            pw tile spans a PAIR of groups, q = g%2)
  i+3  ACT: ws [128,8] bf16 = sigmoid(pw + b2) once per pair
  i+4  PE : po[32*slot:+1, 0:128] = ws_cA.T @ nat_a + ws_cB.T @ nat_b
            (psum row per batch at partitions {0,32,64,96}; [b,e] layout)
  i+6  DVE: osb half = copy(po pair tile [128,128])
  i+8  DMA: 8 output rows (2 pairs) sbuf -> DRAM, partition-strided AP
"""

import sys

sys.path.insert(0, "/opt/trn_rl_repo")

import numpy as np
import ml_dtypes

import concourse.bass as bass
import concourse.mybir as mybir
from concourse.tile import TileContext
from concourse.bass_utils import run_bass_kernel_spmd

F32 = mybir.dt.float32
BF16 = mybir.dt.bfloat16
AF = mybir.ActivationFunctionType
ALU = mybir.AluOpType

B, S, E, A = 2048, 200, 128, 64
NCORES = 8
BL = B // NCORES  # 256 batches per core
G = 2  # batches per group (stacked in partition halves)
NG = BL // G  # 128 groups
DG = 16  # batches per DMA granule
GPG = DG // G  # groups per granule (8)
NDG = BL // DG  # 16 granules
SA, SB = 128, S - 128  # s-chunk sizes (128 + 72)


def build_graph() -> bass.Bass:
    nc = bass.Bass()

    xt_d = nc.declare_dram_parameter("xt", [E, BL * S], BF16, isOutput=False)
    na_d = nc.declare_dram_parameter("nat_a", [SA, BL * E], BF16, isOutput=False)
    nb_d = nc.declare_dram_parameter("nat_b", [SB, BL * E], BF16, isOutput=False)
    wb_d = nc.declare_dram_parameter("wb", [E, BL * A], BF16, isOutput=False)
    w2s_d = nc.declare_dram_parameter("w2s", [128, 2], BF16, isOutput=False)
    b2c_d = nc.declare_dram_parameter("b2c", [128, 1], F32, isOutput=False)
    csb2_d = nc.declare_dram_parameter("csb2", [128, NG], F32, isOutput=False)
    out_d = nc.declare_dram_parameter("out", [BL, E], F32, isOutput=True)

    with TileContext(nc) as tc:
        with (
            tc.tile_pool(name="consts", bufs=1) as cpool,
            tc.tile_pool(name="xtp", bufs=3) as xtpool,
            tc.tile_pool(name="nap", bufs=3) as napool,
            tc.tile_pool(name="nbp", bufs=3) as nbpool,
            tc.tile_pool(name="wbp", bufs=3) as wbpool,
            tc.tile_pool(name="hs", bufs=3) as hspool,
            tc.tile_pool(name="ws", bufs=3) as wspool,
            tc.tile_pool(name="osb", bufs=3) as osbpool,
            tc.tile_pool(name="ph", bufs=3, space="PSUM") as php,
            tc.tile_pool(name="pw", bufs=2, space="PSUM") as pwp,
            tc.tile_pool(name="po", bufs=3, space="PSUM") as pop,
        ):
            w2s = cpool.tile([128, 2], BF16)
            b2c = cpool.tile([128, 1], F32)
            csb2 = cpool.tile([128, NG], F32)
            nc.sync.dma_start(out=w2s[:], in_=w2s_d[:])
            nc.sync.dma_start(out=b2c[:], in_=b2c_d[:])
            nc.sync.dma_start(out=csb2[:], in_=csb2_d[:])

            gran = {}  # dg -> (xtt, nat, nbt, wbt)
            phs = {}  # g -> ph tile
            hss = {}  # g -> hs tile
            pws = {}  # pair -> pw tile
            wss = {}  # pair -> ws tile
            pos = {}  # pair -> po tile
            osbs = {}  # k -> osb tile (8 batches)

            for i in range(NG + 8):
                # -- granule DMA loads (16 batches ahead of use) --
                if i % GPG == 0 and i < NG:
                    dg = i // GPG
                    xtt = xtpool.tile([E, DG * S], BF16, tag="xt")
                    nat = napool.tile([SA, DG * E], BF16, tag="na")
                    nbt = nbpool.tile([SB, DG * E], BF16, tag="nb")
                    wbt = wbpool.tile([E, DG * A], BF16, tag="wb")
                    sl = slice(dg * DG * S, (dg + 1) * DG * S)
                    se = slice(dg * DG * E, (dg + 1) * DG * E)
                    sa = slice(dg * DG * A, (dg + 1) * DG * A)
                    nc.gpsimd.dma_start(out=xtt[:], in_=xt_d[:, sl])
                    nc.sync.dma_start(out=nat[:], in_=na_d[:, se])
                    nc.gpsimd.dma_start(out=nbt[:], in_=nb_d[:, se])
                    nc.scalar.dma_start(out=wbt[:], in_=wb_d[:, sa])
                    gran[dg] = (xtt, nat, nbt, wbt)

                # -- PE: mm1(i) --
                if i < NG:
                    g = i
                    xtt, _, _, wbt = gran[g // GPG]
                    ph = php.tile([128, S], mybir.dt.float32, tag="ph")
                    for j in range(G):
                        b = (g % GPG) * G + j  # batch idx within granule
                        nc.tensor.matmul(
                            ph[64 * j : 64 * j + 64, :],
                            wbt[:, b * A : (b + 1) * A],
                            xtt[:, b * S : (b + 1) * S],
                            start=True,
                            stop=True,
                        )
                    phs[g] = ph

                # -- DVE: relu(i-1) --
                if 0 <= i - 1 < NG:
                    g = i - 1
                    hs = hspool.tile([128, S], BF16, tag="hs")
                    nc.vector.tensor_scalar(
                        hs[:], phs.pop(g)[:], csb2[:, g : g + 1], 0.0,
                        op0=ALU.add, op1=ALU.max,
                    )
                    hss[g] = hs

                # -- PE: pw(i-2), pair-merged psum tile --
                if 0 <= i - 2 < NG:
                    g = i - 2
                    p, q = g // 2, g % 2
                    if q == 0:
                        pws[p] = pwp.tile([128, 8], mybir.dt.float32, tag="pw", name=f"pw{p}")
                    pw = pws[p]
                    hs = hss.pop(g)
                    nc.tensor.matmul(
                        pw[0:128, 4 * q : 4 * q + 2], hs[:, 0:SA], w2s[:],
                        start=True, stop=True,
                    )
                    nc.tensor.matmul(
                        pw[0:SB, 4 * q + 2 : 4 * q + 4], hs[:, SA:S], w2s[:],
                        start=True, stop=True,
                    )

                # -- ACT: sigmoid per pair, after pw of odd group --
                if 0 <= i - 3 < NG and (i - 3) % 2 == 1:
                    p = (i - 3) // 2
                    ws = wspool.tile([128, 8], BF16, tag="ws")
                    nc.scalar.activation(
                        ws[:], pws.pop(p)[:], AF.Sigmoid, bias=b2c[:, 0:1], scale=1.0
                    )
                    wss[p] = ws

                # -- PE: po(i-4), 4 batches per psum tile: partition {0,64} x col half --
                if 0 <= i - 4 < NG:
                    g = i - 4
                    p, q = g // 2, g % 2
                    m, c = g // 2, g % 2  # po-tile index, column half
                    if c == 0:
                        pos[m] = pop.tile([128, 256], mybir.dt.float32, tag="po", name=f"po{m}")
                    po = pos[m]
                    ws = wss[p]
                    _, nat, nbt, _ = gran[g // GPG]
                    for j in range(G):
                        b = (g % GPG) * G + j
                        nc.tensor.matmul(
                            po[64 * j : 64 * j + 1, c * E : c * E + E],
                            ws[0:SA, 4 * q + j : 4 * q + j + 1],
                            nat[:, b * E : (b + 1) * E],
                            start=True,
                            stop=False,
                        )
                        nc.tensor.matmul(
                            po[64 * j : 64 * j + 1, c * E : c * E + E],
                            ws[0:SB, 4 * q + 2 + j : 4 * q + 3 + j],
                            nbt[:, b * E : (b + 1) * E],
                            start=False,
                            stop=True,
                        )
                    if q == 1:
                        wss.pop(p)

                # -- DVE: drain po tile (4 batches) into osb half --
                if 0 <= i - 6 < NG and (i - 6) % 2 == 1:
                    m = (i - 6) // 2
                    k, half = m // 2, m % 2
                    if half == 0:
                        osbs[k] = osbpool.tile([128, 512], mybir.dt.float32, tag="osb", name=f"osb{k}")
                    nc.vector.tensor_copy(
                        osbs[k][:, half * 256 : half * 256 + 256], pos.pop(m)[:]
                    )

                # -- out DMA: 8 rows per osb tile (2 po tiles) --
                if i >= 10 and (i - 10) % 4 == 0 and (i - 10) // 4 * 8 < BL:
                    k = (i - 10) // 4
                    r0 = k * 8
                    # sbuf [2 part (stride 64), mm, c, e] <-> dram row r=4mm+2c+j
                    dst = out_d[r0 : r0 + 8, :].rearrange(
                        "(mm c j) e -> j mm c e", mm=2, c=2, j=2
                    )
                    nc.scalar.dma_start(out=dst, in_=osbs.pop(k)[0:128:64, :])
    _hoist_excess_waits(nc)
    return nc


# Instructions on engine queues accept only ONE sync-wait command in this
# toolchain (walrus setupSyncWait). Tile's sem assigner sometimes attaches
# more. Hoist the excess onto same-engine NoOps inserted immediately before
# the instruction — identical semantics, the wait just moves one queue slot
# earlier. DMA/Drain/branch instructions are exempt (different lowering).
_WAIT_CAP_EXEMPT = {"InstNoOp"}


def _hoist_excess_waits(nc) -> int:
    k = 0
    for fn in nc.m.functions:
        for bb in fn.blocks:
            il = bb.instructions
            out = []
            changed = False
            for inst in il:
                si = inst.sync_info
                tn = type(inst).__name__
                if si is not None and len(si.on_wait) > 1 and tn not in _WAIT_CAP_EXEMPT:
                    waits = list(si.on_wait)
                    for w in waits[:-1]:
                        nop = mybir.InstNoOp(name=f"W-hoist-{k}")
                        k += 1
                        nop.engine = inst.engine
                        nop.sync_info = mybir.SyncInfo(on_wait=[w], on_update=[])
                        out.append(nop)
                    inst.sync_info = mybir.SyncInfo(
                        on_wait=[waits[-1]], on_update=list(si.on_update)
                    )
                    changed = True
                out.append(inst)
            if changed:
                bb.instructions = out
    return k


_GRAPH_CACHE: dict = {}

# test-harness hooks (harness calls kernel() with defaults; test.py flips TRACE)
TRACE = False
TRACE_TMPDIR = None
LAST_RESULT = None


def kernel(**inputs) -> np.ndarray:
    BF = ml_dtypes.bfloat16
    behaviors = np.asarray(inputs["behaviors"], dtype=np.float32)
    target = np.asarray(inputs["target"], dtype=np.float32)
    W1 = np.asarray(inputs["W1"], dtype=np.float32)
    b1 = np.asarray(inputs["b1"], dtype=np.float32)
    W2 = np.asarray(inputs["W2"], dtype=np.float32)
    b2 = np.asarray(inputs["b2"], dtype=np.float32)

    W1a, W1b, W1c, W1d = W1[0:E], W1[E : 2 * E], W1[2 * E : 3 * E], W1[3 * E :]
    W1ad = W1a + W1d  # [E, A]
    W1bd = W1b - W1d  # [E, A]
    b2f = float(np.asarray(b2).reshape(-1)[0])

    if "nc" not in _GRAPH_CACHE:
        _GRAPH_CACHE["nc"] = build_graph()
    nc = _GRAPH_CACHE["nc"]

    x = behaviors.reshape(NCORES, BL, S, E)
    t = target.reshape(NCORES, BL, E)

    # w2s: [[W2, 0], [0, W2]] so one matmul computes both stacked batches
    w2s = np.zeros((128, 2), dtype=np.float32)
    w2s[0:A, 0] = W2[:, 0]
    w2s[A:128, 1] = W2[:, 0]
    w2s = w2s.astype(BF)
    b2c = np.full((128, 1), b2f, dtype=np.float32)

    in_maps = []
    for i in range(NCORES):
        xi = x[i]  # [BL, S, E] f32
        ti = t[i]  # [BL, E]
        xt = np.ascontiguousarray(xi.transpose(2, 0, 1)).astype(BF).reshape(E, BL * S)
        nat = np.ascontiguousarray(xi.transpose(1, 0, 2)).astype(BF)  # [S, BL, E]
        na = nat[0:SA].reshape(SA, BL * E)
        nb = nat[SA:S].reshape(SB, BL * E)
        # per-batch folded weight: Wb = W1ad + t_col * W1c  -> [E, BL*A]
        wb = W1ad[None, :, :] + ti[:, :, None] * W1c[None, :, :]  # [BL, E, A]
        wb = np.ascontiguousarray(wb.transpose(1, 0, 2)).astype(BF).reshape(E, BL * A)
        # per-batch bias, stacked per group: csb2[j*64+a, g] = csb[2g+j, a]
        csb = ti @ W1bd + b1[None, :]  # [BL, A] f32
        csb2 = np.ascontiguousarray(
            csb.reshape(NG, G, A).transpose(1, 2, 0).reshape(128, NG)
        )
        in_maps.append(
            dict(xt=xt, nat_a=na, nat_b=nb, wb=wb, w2s=w2s, b2c=b2c, csb2=csb2)
        )

    global LAST_RESULT
    kw = {}
    if TRACE:
        kw = dict(trace=True, tmpdir=TRACE_TMPDIR)
    res = run_bass_kernel_spmd(nc, in_maps, core_ids=list(range(NCORES)), **kw)
    LAST_RESULT = res
    out = np.stack([res.results[i]["out"] for i in range(NCORES)], axis=0)
    return out.reshape(B, E).astype(np.float32)


if __name__ == "__main__":
    rng = np.random.default_rng(0)
    ins = dict(
        behaviors=rng.standard_normal((B, S, E), dtype=np.float32),
        target=rng.standard_normal((B, E), dtype=np.float32),
        W1=rng.standard_normal((4 * E, A), dtype=np.float32) * 0.04,
        b1=rng.standard_normal((A,), dtype=np.float32) * 0.04,
        W2=rng.standard_normal((A, 1), dtype=np.float32) * 0.1,
        b2=rng.standard_normal((1,), dtype=np.float32) * 0.1,
    )
    o = kernel(**ins)
    print("kernel out", o.shape, o.dtype, np.abs(o).mean())
